# revision 13
# baseline (speedup 1.0000x reference)
"""CavAttention Trainium2 kernel (fused-DVE rewrite).

Computation (per spatial location (b,h,w), L=5 "cav" slots, 8 heads x 32 dim):
  qkv = x @ w_qkv ; att = softmax_j(mask * q_i.k_j / sqrt(d)) ; o = att @ v ; out = o @ w_out + b_out

Distribution: shard the H axis (48) across the 8 cores (6 each); weights replicated.

Per-core layout: locations (b,h,w) ride the 128 SBUF partitions; (l, head, d)
rides the free axis in bf16 (DVE 2x packed mode: 0.52 ns/elem vs 1.04 at 1x).
Measured DVE cost law: 0.52 ns/elem (2x) + ~150 ns/instruction, so the
attention core is emitted as ~14 big fused ops per 128-location tile instead
of ~52 small ones: one broadcast QK mul [p,i,j,(m d)], a 5-op pairwise d-tree,
mask-bias add, ACT exp (interleaved with the AV stage of the previous tile to
hide the ACT round-trip), j-reduce, reciprocal, softmax normalize, one 4-dim
broadcast AV mul [p,i,j,d,m], and a 3-op j-tree.

The output projection runs transposed: w_out chunks are the PE stationary and
the transposed attention output streams through, producing out^T (c-major) in
PSUM. That kills the per-i bias matmuls (b_out is added on the host) and the
f32 staging copies; out^T is cast to bf16 on ACT (halving output DMA traffic)
and the host transposes back / upcasts.
"""

import numpy as np

B, L, H, W, C = 2, 5, 48, 176, 256
HEADS, DIM_HEAD = 8, 32
INNER = HEADS * DIM_HEAD  # 256
SCALE = DIM_HEAD ** -0.5
NCORES = 8
HP = H // NCORES  # 6 h-planes per core
NBH = B * HP      # 12 (b,h) blocks per core
LOCS = NBH * W    # 2112 locations per core
PTILE = 128       # locations per tile
NTILES = (LOCS + PTILE - 1) // PTILE  # 17

_cached = {}


def _pieces(s, e):
    """Split flat loc range [s,e) into (p0, b, h, w0, w1) pieces within (b,h) blocks."""
    out = []
    cur = s
    while cur < e:
        bh = cur // W
        w0 = cur % W
        w1 = min(W, w0 + (e - cur))
        out.append((cur - s, bh // HP, bh % HP, w0, w1))
        cur += w1 - w0
    return out


def _build_bass():
    import concourse.bass as bass
    import concourse.bacc as bacc
    import concourse.tile as tile
    from concourse import mybir
    from concourse.masks import make_identity

    f32 = mybir.dt.float32
    bf16 = mybir.dt.bfloat16

    nc = bacc.Bacc()
    # x arrives pre-transposed and pre-cast on the host: [cc, c, b, h, l, w]
    xT_d = nc.dram_tensor("xT", [2, 128, B, HP, L, W], bf16, kind="ExternalInput")
    # mask arrives as a pre-computed f32 logit bias (0 valid / -1e4 masked)
    mb_d = nc.dram_tensor("mbias", [B, HP, W, L], f32, kind="ExternalInput")
    wqkv_d = nc.dram_tensor("w_qkv", [C, 3 * INNER], f32, kind="ExternalInput")
    wout_d = nc.dram_tensor("w_out", [INNER, C], f32, kind="ExternalInput")
    # out^T, bf16, pre-bias: element (o, c, b, h, l, w) = out[b, l, h, w, o*128+c]
    outT_d = nc.dram_tensor("outT", [2, 128, B, HP, L, W], bf16, kind="ExternalOutput")

    with tile.TileContext(nc) as tc:
        with (
            tc.tile_pool(name="singles", bufs=1) as singles,
            tc.tile_pool(name="work", bufs=3) as work,
            tc.tile_pool(name="peri", bufs=2) as peri,
            tc.tile_pool(name="ps_t", bufs=1, space="PSUM") as ps_t,
            tc.tile_pool(name="ps_qkv", bufs=2, space="PSUM") as ps_qkv,
            tc.tile_pool(name="ps_o", bufs=1, space="PSUM") as ps_o,
        ):
            # ---- constants.  Tensors touched by PE matmuls are produced by
            #      ONE engine (ACT): PE instructions carry a single
            #      semaphore wait (walrus S3_LW limit).
            #      Order: w_qkv DMA + cast first — it gates proj(0), the head
            #      of the pipeline-fill critical path. ----
            wqkv_l = singles.tile([128, 2, 3 * INNER], f32)
            nc.sync.dma_start(
                out=wqkv_l,
                in_=wqkv_d[:, :].rearrange("(cc p) n -> p cc n", cc=2),
            )
            wqkv_sb = singles.tile([128, 2, 3 * INNER], bf16)
            for cc in range(2):
                nc.scalar.copy(out=wqkv_sb[:, cc], in_=wqkv_l[:, cc])
            ident_l = singles.tile([128, 128], f32)
            make_identity(nc, ident_l)  # gpsimd
            wout_l = singles.tile([128, 2, C], f32)
            nc.sync.dma_start(
                out=wout_l,
                in_=wout_d[:, :].rearrange("(cc p) n -> p cc n", cc=2),
            )

            ident = singles.tile([128, 128], bf16)
            wout_sb = singles.tile([128, 2, C], bf16)

            def setup_tail():
                """ident/wout casts — needed first by stage_b_out(0) in
                iteration 2; emitted after proj(0) so they don't sit ahead of
                the fill-critical qkv psum->sbuf copies in the ACT queue."""
                nc.scalar.copy(out=ident, in_=ident_l)
                nc.scalar.copy(out=wout_sb, in_=wout_l)

            def load(t):
                """DMA in (x already transposed+bf16 on host; partitions = C-chunk)."""
                s = t * PTILE
                e = min(s + PTILE, LOCS)
                P = e - s
                pieces = _pieces(s, e)

                xt = work.tile([128, 2, L, 128], bf16, tag="xt")
                for (p0, b, h, w0, w1) in pieces:
                    for cc in range(2):
                        nc.sync.dma_start(
                            out=xt[:, cc, :, p0:p0 + (w1 - w0)],
                            in_=xT_d[cc, :, b, h, :, w0:w1],
                        )
                mbias = work.tile([128, L], f32, tag="mbias")
                for (p0, b, h, w0, w1) in pieces:
                    nc.sync.dma_start(
                        out=mbias[p0:p0 + (w1 - w0), :],
                        in_=mb_d[b, h, w0:w1, :],
                    )
                return dict(P=P, pieces=pieces, xt=xt, mbias=mbias)

            def proj(st):
                """qkv projection on PE + ACT psum->sbuf cast."""
                P = st["P"]
                xt = st["xt"]
                qkv_bf = work.tile([128, L, 3 * INNER], bf16, tag="qkv_bf")
                for l in range(L):
                    pq = ps_qkv.tile([128, 3 * INNER], f32, tag="psq")
                    for cc in range(2):
                        for (n0, n1) in ((0, 512), (512, 768)):
                            nc.tensor.matmul(
                                out=pq[:P, n0:n1],
                                lhsT=xt[:, cc, l, :P],
                                rhs=wqkv_sb[:, cc, n0:n1],
                                start=(cc == 0),
                                stop=(cc == 1),
                            )
                    nc.scalar.copy(out=qkv_bf[:P, l, :], in_=pq[:P, :])
                st["qkv_bf"] = qkv_bf

            def stage_a1(st):
                """QK^T + mask bias on DVE; kicks off ACT exp.  -> st['am' .. ]"""
                P = st["P"]
                qkv_bf = st["qkv_bf"]
                # q[p, i, (m d)] bcast over j;  k[p, j, (m d)] bcast over i
                q_v = qkv_bf[:P, :, 0:INNER].unsqueeze(2).broadcast_to([P, L, L, INNER])
                k_v = qkv_bf[:P, :, INNER:2 * INNER].unsqueeze(1).broadcast_to([P, L, L, INNER])
                qk = peri.tile([128, L * L * HEADS, DIM_HEAD], bf16, tag="qk")
                nc.vector.tensor_mul(
                    qk[:P].rearrange("p (i j m) d -> p i j (m d)", i=L, j=L),
                    q_v, k_v,
                )
                # pairwise d-tree: 32 -> 16 -> 8 -> 4 -> 2 -> 1
                t16 = peri.tile([128, L * L * HEADS, 16], bf16, tag="t16")
                nc.vector.tensor_add(t16[:P], qk[:P, :, 0:16], qk[:P, :, 16:32])
                t8 = peri.tile([128, L * L * HEADS, 8], bf16, tag="t8")
                nc.vector.tensor_add(t8[:P], t16[:P, :, 0:8], t16[:P, :, 8:16])
                t4 = peri.tile([128, L * L * HEADS, 4], bf16, tag="t4")
                nc.vector.tensor_add(t4[:P], t8[:P, :, 0:4], t8[:P, :, 4:8])
                t2 = peri.tile([128, L * L * HEADS, 2], bf16, tag="t2")
                nc.vector.tensor_add(t2[:P], t4[:P, :, 0:2], t4[:P, :, 2:4])
                # final fold in f32 + mask bias (am = A + mbias, bias bcast over i, m)
                A = peri.tile([128, L, L, HEADS], f32, tag="A")
                nc.vector.tensor_add(
                    A[:P],
                    t2[:P, :, 0].rearrange("p (i j m) -> p i j m", i=L, j=L),
                    t2[:P, :, 1].rearrange("p (i j m) -> p i j m", i=L, j=L),
                )
                am = peri.tile([128, L, L, HEADS], f32, tag="am")
                nc.vector.tensor_add(
                    am[:P], A[:P],
                    st["mbias"][:P].unsqueeze(1).unsqueeze(3).broadcast_to([P, L, L, HEADS]),
                )
                # exp on ACT (runs while DVE does the AV stage of tile t-2)
                ee = work.tile([128, L, L, HEADS], bf16, tag="ee")
                nc.scalar.activation(
                    out=ee[:P], in_=am[:P], func=mybir.ActivationFunctionType.Exp,
                    scale=SCALE,
                )
                st["ee"] = ee

            def stage_a2(st):
                """softmax tail: j-sum, reciprocal, normalize -> st['pw']."""
                P = st["P"]
                ee = st["ee"]
                ssum = work.tile([128, L, HEADS], f32, tag="ssum")
                nc.vector.reduce_sum(
                    out=ssum[:P], in_=ee[:P].transpose([0, 1, 3, 2]), axis=mybir.AxisListType.X
                )
                # ~51 ULP approx, ~5x faster than iterative reciprocal; ssum is
                # strictly positive and well inside the safe range
                sinv = work.tile([128, L, HEADS], f32, tag="sinv")
                nc.vector.reciprocal_approx_fast(out=sinv[:P], in_=ssum[:P])
                pw = work.tile([128, L, L, HEADS], bf16, tag="pw")
                nc.vector.tensor_mul(
                    pw[:P], ee[:P],
                    sinv[:P].unsqueeze(2).broadcast_to([P, L, L, HEADS]),
                )
                st["pw"] = pw

            def stage_b_av(st, ia, ib):
                """attention-weighted V for i in [ia, ib): one 4-dim broadcast
                mul + 3-op j-tree.

                V rides in (d, m) order (host-permuted w_qkv columns) so the pw
                broadcast lands on a non-inner dim; w_out rows are host-permuted
                to match."""
                P = st["P"]
                ni = ib - ia
                pw = st["pw"]
                qkv_bf = st["qkv_bf"]
                # v[p, j, d, m] bcast over i;  pw[p, i, j, m] bcast over d
                v4 = (qkv_bf[:P, :, 2 * INNER:3 * INNER]
                      .rearrange("p j (d m) -> p j d m", m=HEADS)
                      .unsqueeze(1).broadcast_to([P, ni, L, DIM_HEAD, HEADS]))
                pw4 = pw[:P, ia:ib].unsqueeze(3).broadcast_to([P, ni, L, DIM_HEAD, HEADS])
                if "av" not in st:
                    st["av"] = peri.tile([128, L, L, INNER], bf16, tag="av", name="av")
                    st["s2"] = peri.tile([128, L, 2, INNER], bf16, tag="s2", name="s2")
                    st["s1"] = peri.tile([128, L, INNER], bf16, tag="s1", name="s1")
                    st["attout"] = work.tile([128, L, INNER], bf16, tag="attout", name="attout")
                av, s2, s1, attout = st["av"], st["s2"], st["s1"], st["attout"]
                nc.vector.tensor_mul(
                    av[:P, ia:ib].rearrange("p i j (d m) -> p i j d m", m=HEADS), v4, pw4,
                )
                # j-tree: 5 -> (2+2) -> 1 (+ leftover j=4)
                nc.vector.tensor_add(s2[:P, ia:ib], av[:P, ia:ib, 0:2], av[:P, ia:ib, 2:4])
                nc.vector.tensor_add(s1[:P, ia:ib], s2[:P, ia:ib, 0], s2[:P, ia:ib, 1])
                nc.vector.tensor_add(attout[:P, ia:ib], s1[:P, ia:ib], av[:P, ia:ib, 4])

            def stage_b_out(st, ia, ib):
                """PE transposes + transposed out-projection + bf16 store, for
                i in [ia, ib)."""
                P = st["P"]
                attout = st["attout"]
                # transpose attout[:, i, cc*128:(cc+1)*128] -> pt[:, i, cc, :P]
                if "pt" not in st:
                    st["pt"] = ps_t.tile([128, L, 2, 128], bf16, tag="pst", name="pt")
                    st["aoti"] = peri.tile([128, L, 2, 128], bf16, tag="aoti", name="aoti")
                    st["osb"] = peri.tile([128, 2, L, 128], bf16, tag="osb", name="osb")
                pt, aoti, osb = st["pt"], st["aoti"], st["osb"]
                for i in range(ia, ib):
                    for cc in range(2):
                        nc.tensor.transpose(
                            pt[:, i, cc, :P],
                            attout[:P, i, cc * 128:(cc + 1) * 128],
                            ident[:P, :P],
                        )
                nc.scalar.copy(out=aoti[:, ia:ib], in_=pt[:, ia:ib])
                # out^T[c_chunk, (i, loc)] = sum_cc wout[cc, c_chunk]^T @ aoti[cc]
                # accumulation groups must stay inside one 2KB PSUM bank
                # (f32 col 512 == i 4), so split i-ranges at 4.
                igroups = [(a, b) for (a, b) in ((ia, min(ib, 4)), (max(ia, 4), ib)) if a < b]
                for o in range(2):
                    po = ps_o.tile([128, L, 128], f32, tag="pso")
                    for (i0, i1) in igroups:
                        for cc in range(2):
                            nc.tensor.matmul(
                                out=po[:, i0:i1, :P],
                                lhsT=wout_sb[:, cc, o * 128:(o + 1) * 128],
                                rhs=aoti[:, i0:i1, cc, :P],
                                start=(cc == 0),
                                stop=(cc == 1),
                            )
                    nc.scalar.copy(out=osb[:, o, ia:ib], in_=po[:, ia:ib])
                for (p0, b, h, w0, w1) in st["pieces"]:
                    for o in range(2):
                        nc.sync.dma_start(
                            out=outT_d[o, :, b, h, ia:ib, w0:w1],
                            in_=osb[:, o, ia:ib, p0:p0 + (w1 - w0)],
                        )

            # 3-deep software pipeline: per iteration t issue
            #   load(t)+proj(t)    DMA + PE qkv + ACT cast
            #   stage_a1(t-1)      DVE qk -> ACT exp
            #   stage_b(t-2)       DVE av (covers the exp round-trip)
            #   stage_a2(t-1)      DVE softmax tail
            #   stage_b_out(t-2)   PE transpose + out-proj; ACT cast; DMA store
            sts = {}
            for t in range(NTILES + 2):
                if t < NTILES:
                    sts[t] = load(t)
                    proj(sts[t])
                    if t == 0:
                        setup_tail()
                if 0 <= t - 1 < NTILES:
                    stage_a1(sts[t - 1])
                if 0 <= t - 2 < NTILES:
                    if t - 2 == NTILES - 1:
                        # last tile: chunk the backend per-i so its PE/ACT/DMA
                        # tail overlaps the tail of the DVE work (pipeline drain)
                        for i in range(L):
                            stage_b_av(sts[t - 2], i, i + 1)
                            stage_b_out(sts[t - 2], i, i + 1)
                    else:
                        stage_b_av(sts[t - 2], 0, 5)
                if 0 <= t - 1 < NTILES:
                    stage_a2(sts[t - 1])
                if 0 <= t - 2 < NTILES:
                    if t - 2 != NTILES - 1:
                        stage_b_out(sts[t - 2], 0, 5)
                    del sts[t - 2]
    nc.finalize()  # Bacc.compile(): legalize multi-wait instructions, alloc regs
    return nc


def get_nc():
    if "nc" not in _cached:
        _cached["nc"] = _build_bass()
    return _cached["nc"]


def make_in_maps(x, mask, w_qkv, w_out, b_out):
    """Host-side shard + repack: x is transposed to [cc, c, b, h, l, w] and
    cast to bf16; the mask becomes an f32 additive logit bias."""
    import ml_dtypes

    x = np.asarray(x, dtype=np.float32)
    mask = np.asarray(mask)
    w_qkv = np.ascontiguousarray(np.asarray(w_qkv), dtype=np.float32)
    w_out = np.ascontiguousarray(np.asarray(w_out), dtype=np.float32)

    # permute V's output columns (m,d)->(d,m) and w_out's rows to match, so
    # the device-side pw broadcast is never on the innermost dim
    wv = w_qkv[:, 2 * INNER:].reshape(C, HEADS, DIM_HEAD).transpose(0, 2, 1).reshape(C, INNER)
    w_qkv = np.ascontiguousarray(np.concatenate([w_qkv[:, :2 * INNER], wv], axis=1))
    w_out = np.ascontiguousarray(
        w_out.reshape(HEADS, DIM_HEAD, C).transpose(1, 0, 2).reshape(INNER, C)
    )

    # [B, L, H, W, C] -> [C, B, H, L, W] -> [2, 128, B, H, L, W] bf16
    xT = np.transpose(x, (4, 0, 2, 1, 3)).astype(ml_dtypes.bfloat16)
    xT = np.ascontiguousarray(xT.reshape(2, 128, B, H, L, W))
    # [B, H, W, 1, L] -> f32 bias [B, H, W, L]
    mb = np.ascontiguousarray(
        np.where(mask[:, :, :, 0, :] != 0, 0.0, -1.0e4).astype(np.float32)
    )

    in_maps = []
    for k in range(NCORES):
        h0, h1 = k * HP, (k + 1) * HP
        in_maps.append({
            "xT": np.ascontiguousarray(xT[:, :, :, h0:h1]),
            "mbias": np.ascontiguousarray(mb[:, h0:h1]),
            "w_qkv": w_qkv,
            "w_out": w_out,
        })
    return in_maps


def assemble_out(results, b_out):
    """Host-side unshard: out^T bf16 [2, 128, B, HP, L, W] per core ->
    full f32 [B, L, H, W, C] (+ b_out)."""
    outT = np.concatenate([r["outT"] for r in results], axis=3)  # [2,128,B,H,L,W]
    out = np.transpose(outT, (2, 4, 3, 5, 0, 1)).reshape(B, L, H, W, C)
    return out.astype(np.float32) + np.asarray(b_out, dtype=np.float32)


def kernel(x, mask, w_qkv, w_out, b_out):
    from concourse.bass_utils import run_bass_kernel_spmd

    nc = get_nc()
    in_maps = make_in_maps(x, mask, w_qkv, w_out, b_out)
    res = run_bass_kernel_spmd(nc, in_maps, core_ids=list(range(NCORES)))
    return assemble_out(res.results, b_out)
